# revision 17
# baseline (speedup 1.0000x reference)
"""Trainium2 Bass kernel for nn_Decoder_55688545960558.

Hierarchical-attention GRU decoder step, fp8-accelerated:
  word-level Bahdanau attention over (B,T,S,D) encoder outputs,
  masked GRU scan over T turns, utterance-level Bahdanau attention,
  decoder GRU step on [ctx_vec ; embed(x)].

Sharding: pure data-parallel over batch (64 -> 8 per core), no
collectives. All large matmuls run in fp8e4 with DoubleRow perf mode
(two 128-row k-chunks contracted per instruction at 0.5 cycles/column)
accumulating in fp32 PSUM. Weights are host-scaled by 64 (candidate
gate 128, emb half 8/16) so their 0.02-std values sit in e4m3's normal
range; the 1/64 is folded into the activation `scale`. dec_rec_kernel
stays bf16 (its error feeds the output gates unsmoothed). The encoder
stream is dual-loaded: fp8 for the PE score matmuls, bf16 for the DVE
softmax-weighted sum (fp8 operands force DVE 1x mode).

Layouts keep the feature dim on partitions end-to-end; k-chunk pairs
are adjacent in the free dim so every DoubleRow rhs/lhsT slice is a
3-dim AP. The single output transpose goes through the PE.
"""

from contextlib import ExitStack

import numpy as np
import ml_dtypes

import concourse.bass as bass
import concourse.mybir as mybir
import concourse.tile as tile
from concourse import bacc
from concourse.bass_utils import run_bass_kernel_spmd

F32 = mybir.dt.float32
BF16 = mybir.dt.bfloat16
F8 = mybir.dt.float8e4
AF = mybir.ActivationFunctionType
OP = mybir.AluOpType
AX = mybir.AxisListType
DR = mybir.MatmulPerfMode.DoubleRow

NCORES = 8
B = 64            # global batch
BL = B // NCORES  # batches per core (8)
T = 10
S = 50
R = T * S         # 500
D = 1024
U = 1024
C = D // 128      # 8 chunks of the feature dim
CP = C // 2       # 4 chunk pairs for DoubleRow
G3 = 3 * D        # 3072
WS = 64.0         # weight fp8 scale
IWS = 1.0 / WS
GS = 0.5 / WS     # gate activation scale (sigmoid-via-tanh halving folded)
DEBUG = False     # adds intermediate DRAM dumps for bring-up


def _bcast_mid(ap, n):
    """Insert a 0-stride broadcast dim of size n as dim 1 (after partitions)."""
    return bass.AP(tensor=ap.tensor, offset=ap.offset,
                   ap=[ap.ap[0], [0, n]] + list(ap.ap[1:]))


def _bcast_last(ap, n):
    """Append a 0-stride broadcast dim of size n as the innermost dim."""
    return bass.AP(tensor=ap.tensor, offset=ap.offset,
                   ap=list(ap.ap) + [[0, n]])


def build():
    nc = bacc.Bacc("TRN2", target_bir_lowering=False, debug=False,
                   num_devices=NCORES)

    def din(name, shape, dt):
        return nc.dram_tensor(name, list(shape), dt, kind="ExternalInput").ap()

    ins = {}
    ins["enc8"] = din("enc8", [BL, 128, CP, 2, R], F8)
    ins["encb"] = din("encb", [BL, 128, C, R], BF16)
    ins["hid8"] = din("hid8", [128, CP, 2, BL], F8)
    ins["hidT_b"] = din("hidT_b", [128, C, BL], BF16)
    ins["hidT_f"] = din("hidT_f", [128, C, BL], F32)
    ins["xemb8"] = din("xemb8", [128, CP, 2, BL], F8)
    ins["w1w"] = din("w1w8", [128, CP, 2, U], F8)
    ins["w2w"] = din("w2w8", [128, CP, 2, U], F8)
    ins["vw"] = din("vw8", [128, CP, 2, 128], F8)
    ins["w1u"] = din("w1u8", [128, CP, 2, U], F8)
    ins["w2u"] = din("w2u8", [128, CP, 2, U], F8)
    ins["vu"] = din("vu8", [128, CP, 2, 128], F8)
    ins["ctxk"] = din("ctxk8", [128, CP, 2, G3], F8)
    ins["ctxrk"] = din("ctxrk8", [128, CP, 2, G3], F8)
    ins["deckA"] = din("deckA8", [128, CP, 2, G3], F8)
    ins["deckB"] = din("deckB8", [128, CP, 2, G3], F8)
    ins["decrk"] = din("decrk", [C, 128, G3], BF16)   # chunk-major, bf16 x64
    ins["qb_w"] = din("qb_w", [128, C], F32)          # true scale
    ins["qb_u64"] = din("qb_u64", [128, C], F32)      # x64 scale
    ins["cbx"] = din("cbx", [128, 3 * C], F32)        # x64 (cand x128)
    ins["cb1h"] = din("cb1h_b", [1, D], BF16)         # x128
    ins["dbx"] = din("dbx", [128, 3 * C], F32)
    ins["db1h"] = din("db1h_b", [1, D], BF16)
    ins["mask"] = din("mask_t", [128, T, BL], BF16)   # pre-scaled by -0.5
    ins["ones"] = din("ones_b", [1, BL], BF16)
    ins["ident"] = din("ident", [128, 128], F32)

    ins["out"] = nc.dram_tensor("out", [BL, D], F32, kind="ExternalOutput").ap()
    if DEBUG:
        dbg = {}
        for name, shape, dt in [
                ("qsb", [128, C, BL], F32), ("ctxT", [128, C, BL, T], BF16),
                ("seq", [128, C, BL, T], BF16), ("qu64", [128, C, BL], F32),
                ("cvn", [128, C, BL], F32), ("hmd", [128, 3, C, BL], F32),
                ("xmd", [128, 3, C, BL], F32), ("stT", [128, C, BL], F32),
                ("xg", [128, 3, C, BL, T], F32)]:
            dbg[name] = nc.dram_tensor("dbg_" + name, shape, dt,
                                       kind="ExternalOutput").ap()
        ins["_dbg"] = dbg

    with tile.TileContext(nc) as tc:
        _emit(nc, tc, ins)
    nc.compile()
    return nc


def _emit(nc, tc, ins):
    es = ExitStack()

    pers = es.enter_context(tc.tile_pool(name="pers", bufs=1))
    slot1 = es.enter_context(tc.tile_pool(name="slot1", bufs=1))   # ctxk -> deckA
    slot2 = es.enter_context(tc.tile_pool(name="slot2", bufs=1))   # ctxrk -> deckB
    wsmall = es.enter_context(tc.tile_pool(name="wsmall", bufs=1))  # word -> utt
    st8p = es.enter_context(tc.tile_pool(name="st8p", bufs=3))     # enc8 stream
    stbp = es.enter_context(tc.tile_pool(name="stbp", bufs=3))     # encb stream
    drkp = es.enter_context(tc.tile_pool(name="drkp", bufs=3))     # decrk stream
    tanhp = es.enter_context(tc.tile_pool(name="tanhp", bufs=6))
    s1small = es.enter_context(tc.tile_pool(name="s1small", bufs=2))
    prodp = es.enter_context(tc.tile_pool(name="prodp", bufs=2))
    gtmp = es.enter_context(tc.tile_pool(name="gtmp", bufs=3))
    hstate = es.enter_context(tc.tile_pool(name="hstate", bufs=2))
    s4tmp = es.enter_context(tc.tile_pool(name="s4tmp", bufs=1))

    def ld(pool, dram_ap, shape, dt, name, chunked=False):
        t = pool.tile(list(shape), dt, tag=name, name=name)
        if chunked:
            for k in range(shape[1]):
                nc.sync.dma_start(out=t[:, k], in_=dram_ap[:, k])
        else:
            nc.sync.dma_start(out=t[:], in_=dram_ap)
        return t

    # ---- critical-path DMA order on the sync queue
    hid8_s = ld(pers, ins["hid8"], [128, CP, 2, BL], F8, "hid8")
    hidT_b = ld(pers, ins["hidT_b"], [128, C, BL], BF16, "hidT_b")
    qbw_s = ld(pers, ins["qb_w"], [128, C], F32, "qbw")
    vw_s = ld(pers, ins["vw"], [128, CP, 2, 128], F8, "vw")
    w2w_s = wsmall.tile([128, CP, 2, U], F8, tag="wB", name="w2w")
    w1w_s = wsmall.tile([128, CP, 2, U], F8, tag="wA", name="w1w")
    enc8_b0 = st8p.tile([128, CP, 2, R], F8, tag="st8", name="enc8_0")
    for kp in range(CP):
        nc.sync.dma_start(out=w2w_s[:, kp], in_=ins["w2w"][:, kp])
        nc.sync.dma_start(out=w1w_s[:, kp], in_=ins["w1w"][:, kp])
        nc.sync.dma_start(out=enc8_b0[:, kp], in_=ins["enc8"][0][:, kp])
    enc8_tiles = [enc8_b0]
    encb_tiles = [ld(stbp, ins["encb"][0], [128, C, R], BF16, "stb")]

    # weight chunks interleaved between enc batches on the sync queue
    later = []  # list of (tile, dram, chunk_idx or None)
    w1u_s = wsmall.tile([128, CP, 2, U], F8, tag="wA", name="w1u")
    w2u_s = wsmall.tile([128, CP, 2, U], F8, tag="wB", name="w2u")
    vu_s = pers.tile([128, CP, 2, 128], F8, tag="vu", name="vu")
    ctxk_s = slot1.tile([128, CP, 2, G3], F8, tag="slot1", name="ctxk")
    ctxrk_s = slot2.tile([128, CP, 2, G3], F8, tag="slot2", name="ctxrk")
    for kp in range(CP):
        later.append((w1u_s, ins["w1u"], kp))
    for kp in range(CP):
        later.append((w2u_s, ins["w2u"], kp))
    later.append((vu_s, ins["vu"], None))
    for kp in range(CP):
        later.append((ctxk_s, ins["ctxk"], kp))
    for kp in range(CP):
        later.append((ctxrk_s, ins["ctxrk"], kp))
    li = 0

    def later_dma(n):
        nonlocal li
        for _ in range(n):
            if li < len(later):
                tl, dr, k = later[li]
                if k is None:
                    nc.sync.dma_start(out=tl[:], in_=dr)
                else:
                    nc.sync.dma_start(out=tl[:, k], in_=dr[:, k])
                li += 1

    for b in range(1, BL):
        enc8_tiles.append(ld(st8p, ins["enc8"][b], [128, CP, 2, R], F8, "st8"))
        encb_tiles.append(ld(stbp, ins["encb"][b], [128, C, R], BF16, "stb"))
        later_dma(2 if b < 4 else 3)
    later_dma(len(later))
    # decrk chunks + stage-4 weights + smalls behind everything
    decrk_tiles = []
    for k in range(C):
        dk = drkp.tile([128, G3], BF16, tag="drk", name=f"decrk{k}")
        nc.sync.dma_start(out=dk[:], in_=ins["decrk"][k])
        decrk_tiles.append(dk)
    hidT_f = ld(pers, ins["hidT_f"], [128, C, BL], F32, "hidT_f")
    xemb8_s = ld(pers, ins["xemb8"], [128, CP, 2, BL], F8, "xemb8")
    qbu_s = ld(pers, ins["qb_u64"], [128, C], F32, "qbu")
    cbx_s = ld(pers, ins["cbx"], [128, 3 * C], F32, "cbx")
    cb1h_s = ld(pers, ins["cb1h"], [1, D], BF16, "cb1h")
    dbx_s = ld(pers, ins["dbx"], [128, 3 * C], F32, "dbx")
    db1h_s = ld(pers, ins["db1h"], [1, D], BF16, "db1h")
    mask_s = ld(pers, ins["mask"], [128, T, BL], BF16, "mask")
    ones_s = ld(pers, ins["ones"], [1, BL], BF16, "ones")
    ident_s = ld(pers, ins["ident"], [128, 128], F32, "ident")

    # cross-stage activation tensors
    ctxT_b = pers.tile([128, C, BL, T], BF16, tag="ctxT_b")
    ctx8 = pers.tile([128, CP, 2, BL * T], F8, tag="ctx8")
    seq_b = pers.tile([128, C, BL, T], BF16, tag="seq_b")
    seq8 = pers.tile([128, CP, 2, BL * T], F8, tag="seq8")
    qsb = pers.tile([128, C, BL], F32, tag="qsb")
    qu64 = pers.tile([128, C, BL], F32, tag="qu64")
    hmd_sb = pers.tile([128, 3, C, BL], F32, tag="hmd_sb")

    # =================== STAGE 1: word attention ===================
    with tc.tile_pool(name="ps_sc", bufs=2, space="PSUM") as ps_sc, \
         tc.tile_pool(name="pq", bufs=1, space="PSUM") as pq, \
         tc.tile_pool(name="ps_v", bufs=1, space="PSUM") as ps_v:

        def emit_score_pair(enc8_b, mp):
            ps = ps_sc.tile([128, 2, 512], F32, tag="ps")
            for j in range(2):
                m = 2 * mp + j
                for kp in range(CP):
                    nc.tensor.matmul(
                        out=ps[:, j, 0:R],
                        lhsT=w1w_s[:, kp, :, m * 128:(m + 1) * 128],
                        rhs=enc8_b[:, kp], start=(kp == 0),
                        stop=(kp == CP - 1), perf_mode=DR)
            return ps

        def emit_tanh_pair(ps, mp, b):
            th = tanhp.tile([128, 2, R], F8, tag="th")
            for j in range(2):
                m = 2 * mp + j
                nc.scalar.activation(out=th[:, j], in_=ps[:, j, 0:R],
                                     func=AF.Tanh,
                                     bias=qsb[:, m, b:b + 1], scale=IWS)
            return th

        for b in range(BL):
            enc8_b = enc8_tiles[b]
            ths = []
            if b == 0:
                # first two score pair-groups, then the queries (PE reaches
                # them as w2w lands), then their tanhs
                pss = [emit_score_pair(enc8_b, mp) for mp in range(2)]
                p_q = pq.tile([128, C, BL], F32)
                for m in range(C):
                    for kp in range(CP):
                        nc.tensor.matmul(
                            out=p_q[:, m],
                            lhsT=w2w_s[:, kp, :, m * 128:(m + 1) * 128],
                            rhs=hid8_s[:, kp], start=(kp == 0),
                            stop=(kp == CP - 1), perf_mode=DR)
                # qsb = p_q/64 + qb (true scale), one fused op
                nc.vector.scalar_tensor_tensor(
                    out=qsb[:], in0=p_q[:], scalar=IWS,
                    in1=_bcast_last(qbw_s[:], BL), op0=OP.mult, op1=OP.add)
                ths = [emit_tanh_pair(ps, mp, 0) for mp, ps in enumerate(pss)]
                for mp in range(2, CP):
                    ths.append(emit_tanh_pair(emit_score_pair(enc8_b, mp), mp, 0))
            else:
                for mp in range(CP):
                    ths.append(emit_tanh_pair(emit_score_pair(enc8_b, mp), mp, b))
            # v-score with column-replicated V (DoubleRow over m-pairs)
            psc = ps_v.tile([128, R], F32, tag="psc")
            for mp in range(CP):
                nc.tensor.matmul(out=psc[:], lhsT=vw_s[:, mp], rhs=ths[mp][:],
                                 start=(mp == 0), stop=(mp == CP - 1),
                                 perf_mode=DR)
            e = s1small.tile([128, T, S], BF16, tag="e")
            nc.scalar.activation(out=e[:], in_=psc.rearrange("p (t s) -> p t s", s=S),
                                 func=AF.Exp, scale=IWS)
            rs = s1small.tile([128, T], F32, tag="rs")
            nc.vector.reduce_sum(out=rs[:], in_=e[:], axis=AX.X)
            rc = s1small.tile([128, T], F32, tag="rc")
            nc.vector.reciprocal(out=rc[:], in_=rs[:])
            # weighted sum: one big mult, one tree level, one reduce
            encb_b = encb_tiles[b]
            pr = prodp.tile([128, C, T, S], BF16, tag="pr")
            nc.vector.tensor_tensor(
                out=pr[:], in0=encb_b.rearrange("p c (t s) -> p c t s", s=S),
                in1=_bcast_mid(e[:], C), op=OP.mult)
            prh = prodp.tile([128, C, T, S // 2], BF16, tag="prh")
            nc.vector.tensor_tensor(out=prh[:], in0=pr[:, :, :, 0:S // 2],
                                    in1=pr[:, :, :, S // 2:S], op=OP.add)
            cus = s1small.tile([128, C, T], BF16, tag="cus")
            with nc.allow_low_precision(reason="softmax-weighted mean; bf16 ok"):
                nc.vector.reduce_sum(out=cus[:], in_=prh[:], axis=AX.X)
            nc.vector.tensor_tensor(out=ctxT_b[:, :, b, :], in0=cus[:],
                                    in1=_bcast_mid(rc[:], C), op=OP.mult)
    # fp8 copy of ctx for stage-2 DoubleRow rhs
    nc.scalar.activation(
        out=ctx8.rearrange("p cp two (b t) -> p (cp two) b t", b=BL),
        in_=ctxT_b[:], func=AF.Copy)

    # =================== STAGE 2: context GRU ===================
    # xm for all steps: ctx_in @ ctx_kernel (+ gate biases), fp8 DoubleRow
    xg = pers.tile([128, 3, C, BL, T], F32, tag="xg")
    with tc.tile_pool(name="ps_xm", bufs=6, space="PSUM") as ps_xm:
        for g in range(3):
            for half in range(2):
                pxm = ps_xm.tile([128, 4, BL * T], F32, tag="pxm")
                for cc in range(4):
                    c = half * 4 + cc
                    col0 = g * D + c * 128
                    for kp in range(CP):
                        nc.tensor.matmul(out=pxm[:, cc],
                                         lhsT=ctxk_s[:, kp, :, col0:col0 + 128],
                                         rhs=ctx8[:, kp], start=(kp == 0),
                                         stop=(kp == CP - 1), perf_mode=DR)
                for cc in range(4):
                    c = half * 4 + cc
                    nc.vector.tensor_tensor(
                        out=xg[:, g, c],
                        in0=pxm[:, cc].rearrange("p (b t) -> p b t", t=T),
                        in1=_bcast_last(_bcast_last(cbx_s[:, g * C + c], BL), T),
                        op=OP.add)

    # stage-4 weights stream behind (slot rotation frees ctxk/ctxrk tags later)
    deckA_s = ld(slot1, ins["deckA"], [128, CP, 2, G3], F8, "slot1", chunked=True)

    h_f = None
    ps_hmd = es.enter_context(tc.tile_pool(name="ps_hmd", bufs=2, space="PSUM"))

    with tc.tile_pool(name="ps_hm", bufs=2, space="PSUM") as ps_hm, \
         tc.tile_pool(name="h8p", bufs=2) as h8p:
        h8 = None
        for t in range(T):
            phm = ps_hm.tile([128, 3, C, BL], F32, tag="phm")
            for g in (1, 0, 2):   # z,r first (merged tanh); cand last
                for c in range(C):
                    col0 = g * D + c * 128
                    if t > 0:
                        for kp in range(CP):
                            nc.tensor.matmul(out=phm[:, g, c],
                                             lhsT=ctxrk_s[:, kp, :, col0:col0 + 128],
                                             rhs=h8[:, kp], start=(kp == 0),
                                             stop=(kp == CP - 1 and g != 2),
                                             perf_mode=DR)
                    if g == 2:
                        # += ctx_bias[1] h-part (rank-1, x128 host-scaled)
                        nc.tensor.matmul(out=phm[:, g, c],
                                         lhsT=cb1h_s[:, c * 128:(c + 1) * 128],
                                         rhs=ones_s[:], start=(t == 0),
                                         stop=True)
            if t < C:
                # fill the PE gate-chain wait with stage-4 hm_dec chunk t
                k = t
                dk = decrk_tiles[k]
                phmd_k = ps_hmd.tile([128, 3, C, BL], F32, tag="phmd_k")
                for g in range(3):
                    for c in range(C):
                        col0 = g * D + c * 128
                        nc.tensor.matmul(out=phmd_k[:, g, c],
                                         lhsT=dk[:, col0:col0 + 128],
                                         rhs=hidT_b[:, k], start=True,
                                         stop=(k != C - 1 or g != 2))
                        if k == C - 1 and g == 2:
                            nc.tensor.matmul(out=phmd_k[:, g, c],
                                             lhsT=db1h_s[:, c * 128:(c + 1) * 128],
                                             rhs=ones_s[:], start=False, stop=True)
                if k == 0:
                    nc.vector.tensor_copy(out=hmd_sb[:], in_=phmd_k[:])
                else:
                    nc.vector.tensor_tensor(out=hmd_sb[:], in0=hmd_sb[:],
                                            in1=phmd_k[:], op=OP.add)
            if t == 5:
                pqu_es = ExitStack()
                pqu = pqu_es.enter_context(
                    tc.tile_pool(name="pqu", bufs=1, space="PSUM"))
                p_qu = pqu.tile([128, C, BL], F32)
            if t in (5, 6):
                # utt query in late-GRU PE gaps; qu stays x64-scaled
                for m in range((t - 5) * 4, (t - 4) * 4):
                    for kp in range(CP):
                        nc.tensor.matmul(
                            out=p_qu[:, m],
                            lhsT=w2u_s[:, kp, :, m * 128:(m + 1) * 128],
                            rhs=hid8_s[:, kp], start=(kp == 0),
                            stop=(kp == CP - 1), perf_mode=DR)
                if t == 6:
                    nc.vector.tensor_tensor(out=qu64[:], in0=p_qu[:],
                                            in1=_bcast_last(qbu_s[:], BL),
                                            op=OP.add)
                    pqu_es.close()
            # merged z/r gate: one add, one tanh over [128, 2, C, BL]
            mask_bc = _bcast_mid(mask_s[:, t, :], C)
            tzr = gtmp.tile([128, 2, C, BL], F32, tag="tzr")
            if t == 0:
                nc.scalar.activation(out=tzr[:], in_=xg[:, 0:2, :, :, 0],
                                     func=AF.Tanh, scale=GS)
            else:
                zr_in = gtmp.tile([128, 2, C, BL], F32, tag="zr_in")
                nc.vector.tensor_tensor(out=zr_in[:], in0=xg[:, 0:2, :, :, t],
                                        in1=phm[:, 0:2], op=OP.add)
                nc.scalar.activation(out=tzr[:], in_=zr_in[:], func=AF.Tanh,
                                     scale=GS)
            tz, tr = tzr[:, 0], tzr[:, 1]
            # rhh = (tanh_r + 1) * hh  (== 2*r*hh; h-cols host-doubled)
            rhh = gtmp.tile([128, C, BL], F32, tag="rhh")
            nc.vector.scalar_tensor_tensor(out=rhh[:], in0=tr, scalar=1.0,
                                           in1=phm[:, 2], op0=OP.add, op1=OP.mult)
            cin = gtmp.tile([128, C, BL], F32, tag="cin")
            nc.vector.tensor_tensor(out=cin[:], in0=xg[:, 2, :, :, t], in1=rhh[:],
                                    op=OP.add)
            # zcm = (1-z)*mask == (tanh_z - 1) * (-0.5*mask)
            zcm = gtmp.tile([128, C, BL], F32, tag="zcm")
            nc.vector.scalar_tensor_tensor(out=zcm[:], in0=tz, scalar=-1.0,
                                           in1=mask_bc, op0=OP.add, op1=OP.mult)
            h_f2 = hstate.tile([128, C, BL], F32, tag="h_f")
            if t > 0:
                hz1 = gtmp.tile([128, C, BL], F32, tag="hz1")
                nc.vector.tensor_tensor(out=hz1[:], in0=h_f[:], in1=zcm[:],
                                        op=OP.mult)
                hm1 = gtmp.tile([128, C, BL], F32, tag="hm1")
                nc.vector.tensor_tensor(out=hm1[:], in0=h_f[:], in1=hz1[:],
                                        op=OP.subtract)
            cand = gtmp.tile([128, C, BL], F32, tag="cand")
            nc.scalar.activation(out=cand[:], in_=cin[:], func=AF.Tanh, scale=GS)
            if t == 0:
                nc.vector.tensor_tensor(out=h_f2[:], in0=cand[:], in1=zcm[:],
                                        op=OP.mult)
            else:
                t2 = gtmp.tile([128, C, BL], F32, tag="t2")
                nc.vector.tensor_tensor(out=t2[:], in0=cand[:], in1=zcm[:],
                                        op=OP.mult)
                nc.vector.tensor_tensor(out=h_f2[:], in0=hm1[:], in1=t2[:],
                                        op=OP.add)
            nc.vector.tensor_copy(out=seq_b[:, :, :, t], in_=h_f2[:])
            h8 = h8p.tile([128, CP, 2, BL], F8, tag="h8")
            nc.scalar.activation(
                out=h8.rearrange("p cp two b -> p (cp two) b"), in_=h_f2[:],
                func=AF.Copy)
            h_f = h_f2
    # fp8 copy of the full sequence for stage-3 DoubleRow rhs
    nc.scalar.activation(
        out=seq8.rearrange("p cp two (b t) -> p (cp two) b t", b=BL),
        in_=seq_b[:], func=AF.Copy)

    # =================== STAGE 3: utterance attention ===================
    deckB_s = ld(slot2, ins["deckB"], [128, CP, 2, G3], F8, "slot2", chunked=True)

    with tc.tile_pool(name="ps_su", bufs=1, space="PSUM") as ps_su, \
         tc.tile_pool(name="ps_scu", bufs=1, space="PSUM") as ps_scu, \
         tc.tile_pool(name="s3tmp", bufs=2) as s3tmp:
        psu = ps_su.tile([128, C, 128], F32)
        for m in range(C):
            for kp in range(CP):
                nc.tensor.matmul(out=psu[:, m, 0:BL * T],
                                 lhsT=w1u_s[:, kp, :, m * 128:(m + 1) * 128],
                                 rhs=seq8[:, kp], start=(kp == 0),
                                 stop=(kp == CP - 1), perf_mode=DR)
        qn = s3tmp.tile([128, C, BL, T], F32, tag="qn")
        nc.vector.tensor_tensor(
            out=qn[:], in0=psu[:, :, 0:BL * T].rearrange("p m (b t) -> p m b t", t=T),
            in1=_bcast_last(qu64[:], T), op=OP.add)
        thu8 = s3tmp.tile([128, CP, 2, BL * T], F8, tag="thu8")
        nc.scalar.activation(
            out=thu8.rearrange("p mp two (b t) -> p (mp two) b t", b=BL),
            in_=qn[:], func=AF.Tanh, scale=IWS)
        pscu = ps_scu.tile([128, BL, T], F32)
        for mp in range(CP):
            nc.tensor.matmul(out=pscu.rearrange("p b t -> p (b t)"),
                             lhsT=vu_s[:, mp], rhs=thu8[:, mp],
                             start=(mp == 0), stop=(mp == CP - 1), perf_mode=DR)
        eu = s3tmp.tile([128, BL, T], BF16, tag="eu")
        nc.scalar.activation(out=eu[:], in_=pscu[:], func=AF.Exp, scale=IWS)
        rsu = s3tmp.tile([128, BL], F32, tag="rsu")
        nc.vector.reduce_sum(out=rsu[:], in_=eu[:], axis=AX.X)
        rcu = s3tmp.tile([128, BL], F32, tag="rcu")
        nc.vector.reciprocal(out=rcu[:], in_=rsu[:])
        pru = s3tmp.tile([128, C, BL, T], BF16, tag="pru")
        nc.vector.tensor_tensor(out=pru[:], in0=seq_b[:],
                                in1=_bcast_mid(eu[:], C), op=OP.mult)
        cvus = s3tmp.tile([128, C, BL], F32, tag="cvus")
        nc.vector.reduce_sum(out=cvus[:], in_=pru[:], axis=AX.X)
        ctxv8 = pers.tile([128, CP, 2, BL], F8, tag="ctxv8")
        cvn = s3tmp.tile([128, C, BL], F32, tag="cvn")
        nc.vector.tensor_tensor(out=cvn[:], in0=cvus[:],
                                in1=_bcast_mid(rcu[:], C), op=OP.mult)
        nc.scalar.activation(
            out=ctxv8.rearrange("p cp two b -> p (cp two) b"), in_=cvn[:],
            func=AF.Copy)
        if DEBUG:
            cvn_keep = pers.tile([128, C, BL], F32, tag="dbg_cvn")
            nc.vector.tensor_copy(out=cvn_keep[:], in_=cvn[:])

    # =================== STAGE 4: decoder GRU step ===================
    with tc.tile_pool(name="ps_xmd", bufs=1, space="PSUM") as ps_xmd, \
         tc.tile_pool(name="ps_out", bufs=1, space="PSUM") as ps_out:
        pxmd = ps_xmd.tile([128, 3, C, BL], F32)
        for g in range(3):
            for c in range(C):
                col0 = g * D + c * 128
                for kp in range(2 * CP):
                    if kp < CP:
                        lhsT = deckA_s[:, kp, :, col0:col0 + 128]
                        rhs = ctxv8[:, kp]
                    else:
                        lhsT = deckB_s[:, kp - CP, :, col0:col0 + 128]
                        rhs = xemb8_s[:, kp - CP]
                    nc.tensor.matmul(out=pxmd[:, g, c], lhsT=lhsT, rhs=rhs,
                                     start=(kp == 0), stop=(kp == 2 * CP - 1),
                                     perf_mode=DR)
        xmd_sb = s4tmp.tile([128, 3, C, BL], F32, tag="xmd_sb")
        nc.vector.tensor_tensor(
            out=xmd_sb[:], in0=pxmd[:],
            in1=_bcast_last(dbx_s.rearrange("p (g c) -> p g c", g=3), BL),
            op=OP.add)

        tzr4 = s4tmp.tile([128, 2, C, BL], F32, tag="tzr4")
        zr_in4 = s4tmp.tile([128, 2, C, BL], F32, tag="zr_in4")
        nc.vector.tensor_tensor(out=zr_in4[:], in0=xmd_sb[:, 0:2],
                                in1=hmd_sb[:, 0:2], op=OP.add)
        nc.scalar.activation(out=tzr4[:], in_=zr_in4[:], func=AF.Tanh, scale=GS)
        rhh = s4tmp.tile([128, C, BL], F32, tag="rhh4")
        nc.vector.scalar_tensor_tensor(out=rhh[:], in0=tzr4[:, 1], scalar=1.0,
                                       in1=hmd_sb[:, 2], op0=OP.add, op1=OP.mult)
        cin = s4tmp.tile([128, C, BL], F32, tag="cin4")
        nc.vector.tensor_tensor(out=cin[:], in0=xmd_sb[:, 2], in1=rhh[:], op=OP.add)
        cand = s4tmp.tile([128, C, BL], F32, tag="cand4")
        nc.scalar.activation(out=cand[:], in_=cin[:], func=AF.Tanh, scale=GS)
        zcm = s4tmp.tile([128, C, BL], F32, tag="zcm4")
        nc.vector.tensor_scalar(out=zcm[:], in0=tzr4[:, 0], scalar1=-1.0,
                                scalar2=-0.5, op0=OP.add, op1=OP.mult)
        d1 = s4tmp.tile([128, C, BL], F32, tag="d14")
        nc.vector.tensor_tensor(out=d1[:], in0=cand[:], in1=hidT_f[:],
                                op=OP.subtract)
        d2 = s4tmp.tile([128, C, BL], F32, tag="d24")
        nc.vector.tensor_tensor(out=d2[:], in0=d1[:], in1=zcm[:], op=OP.mult)
        stT = s4tmp.tile([128, C, BL], F32, tag="stT")
        nc.vector.tensor_tensor(out=stT[:], in0=hidT_f[:], in1=d2[:], op=OP.add)

        out_sb = s4tmp.tile([BL, D], F32, tag="out_sb")
        for c in range(C):
            po = ps_out.tile([BL, 128], F32, tag="po")
            nc.tensor.transpose(out=po[:], in_=stT[:, c], identity=ident_s[:])
            nc.vector.tensor_copy(out=out_sb[:, c * 128:(c + 1) * 128], in_=po[:])
        nc.sync.dma_start(out=ins["out"], in_=out_sb[:])
        if DEBUG:
            dbg = ins["_dbg"]
            nc.sync.dma_start(out=dbg["qsb"], in_=qsb[:])
            nc.sync.dma_start(out=dbg["ctxT"], in_=ctxT_b[:])
            nc.sync.dma_start(out=dbg["seq"], in_=seq_b[:])
            nc.sync.dma_start(out=dbg["qu64"], in_=qu64[:])
            nc.sync.dma_start(out=dbg["hmd"], in_=hmd_sb[:])
            nc.sync.dma_start(out=dbg["xmd"], in_=xmd_sb[:])
            nc.sync.dma_start(out=dbg["stT"], in_=stT[:])
            nc.sync.dma_start(out=dbg["xg"], in_=xg[:])
            nc.sync.dma_start(out=dbg["cvn"], in_=cvn_keep[:])

    es.close()


# ---------------------------------------------------------------------------
# Host side
# ---------------------------------------------------------------------------

_NC_CACHE = {}


def _get_nc():
    if "prog" not in _NC_CACHE:
        _NC_CACHE["prog"] = build()
    return _NC_CACHE["prog"]


E4 = ml_dtypes.float8_e4m3


def _f8(a, s=1.0):
    return np.ascontiguousarray(
        np.clip(np.asarray(a, np.float32) * s, -240.0, 240.0).astype(E4))


def _bf(a):
    return np.ascontiguousarray(np.asarray(a).astype(ml_dtypes.bfloat16))


def _f32(a):
    return np.ascontiguousarray(np.asarray(a).astype(np.float32))


def _chunked_T(w):
    """[D_in, N] -> [128, D_in//128, N]: row-chunked for per-k lhsT tiles."""
    d_in, n = w.shape
    return np.ascontiguousarray(w.reshape(d_in // 128, 128, n).transpose(1, 0, 2))


def _pairs(a):
    """[128, C, N] -> [128, C//2, 2, N]."""
    p, c, n = a.shape
    return a.reshape(p, c // 2, 2, n)


def prepare_in_maps(inputs):
    x = np.asarray(inputs["x"]).astype(np.int64).reshape(B)
    hidden = _f32(inputs["hidden"])                        # [64, 1024]
    enc = _f32(inputs["encoder_outputs"])                  # [64, 10, 50, 1024]
    maskf = np.asarray(inputs["context_mask"]).astype(np.float32)  # [64, 10]
    emb = np.asarray(inputs["embed_table"])                # [V, 1024]

    x_emb = emb[x].astype(np.float32)                      # [64, 1024]

    def tmajor(a2d):  # [B, D] -> [128, C, B]
        return np.ascontiguousarray(
            a2d.T.reshape(C, 128, a2d.shape[0]).transpose(1, 0, 2))

    def scale_gru(w):
        # x64, candidate-gate columns x128 (sigmoid-via-tanh + r*2hh folding)
        w = np.array(w, np.float32, copy=True)
        w[:, :2 * D] *= WS
        w[:, 2 * D:] *= 2.0 * WS
        return w

    w1w = _f8(_pairs(_chunked_T(np.asarray(inputs["w1_word"], np.float32) * WS)))
    w2w = _f8(_pairs(_chunked_T(np.asarray(inputs["w2_word"], np.float32) * WS)))
    w1u = _f8(_pairs(_chunked_T(np.asarray(inputs["w1_utt"], np.float32) * WS)))
    w2u = _f8(_pairs(_chunked_T(np.asarray(inputs["w2_utt"], np.float32) * WS)))
    ctxk = _f8(_pairs(_chunked_T(scale_gru(inputs["ctx_kernel"]))))
    # recurrent kernels: uniform x64 — the (tanh_r + 1) factor already
    # supplies the 2x on the candidate h-term
    ctxrk = _f8(_pairs(_chunked_T(
        WS * np.asarray(inputs["ctx_rec_kernel"], np.float32))))
    deck_full = np.asarray(inputs["dec_kernel"], np.float32)   # [2048, 3072]
    deckA = _f8(_pairs(_chunked_T(scale_gru(deck_full[:D]))))
    # emb half: x8 (x16 cand) with x8 on the embedding -> product still x64/128
    deckB_sc = np.array(deck_full[D:], np.float32, copy=True)
    deckB_sc[:, :2 * D] *= 8.0
    deckB_sc[:, 2 * D:] *= 16.0
    deckB = _f8(_pairs(_chunked_T(deckB_sc)))
    decrk = _bf((WS * np.asarray(inputs["dec_rec_kernel"], np.float32))
                .reshape(C, 128, G3))

    def vrep(v):   # [U, 1] -> [128, CP, 2, 128] x64, bcast across columns
        vc = np.asarray(v, np.float32).reshape(C, 128).T * WS    # [128, C]
        return _f8(_pairs(np.ascontiguousarray(
            np.broadcast_to(vc[:, :, None], (128, C, 128)))))

    vw = vrep(inputs["v_word"])
    vu = vrep(inputs["v_utt"])

    def mchunk(v):   # [U] -> [128, C]
        return _f32(np.asarray(v, np.float32).reshape(C, 128).T)

    qb_w = mchunk(np.asarray(inputs["b1_word"], np.float32)
                  + np.asarray(inputs["b2_word"], np.float32))
    qb_u64 = mchunk(WS * (np.asarray(inputs["b1_utt"], np.float32)
                          + np.asarray(inputs["b2_utt"], np.float32)))

    cbias = np.asarray(inputs["ctx_bias"], np.float32)      # [2, 3072]
    dbias = np.asarray(inputs["dec_bias"], np.float32)      # [2, 3072]

    def gate_bias(bias2):   # x64 (cand x128) to match the scaled kernels
        return WS * np.concatenate([
            bias2[0, :D] + bias2[1, :D],
            bias2[0, D:2 * D] + bias2[1, D:2 * D],
            2.0 * bias2[0, 2 * D:],
        ])

    cbx = _f32(gate_bias(cbias).reshape(3 * C, 128).T)       # [128, 24]
    dbx = _f32(gate_bias(dbias).reshape(3 * C, 128).T)
    cb1h = _bf((WS * cbias[1, 2 * D:]).reshape(1, D))
    db1h = _bf((WS * dbias[1, 2 * D:]).reshape(1, D))

    ones_b = _bf(np.ones((1, BL), np.float32))
    ident = _f32(np.eye(128, dtype=np.float32))

    enc_r = enc.reshape(B, R, D)

    in_maps = []
    for core in range(NCORES):
        sl = slice(core * BL, (core + 1) * BL)
        enc_t = np.ascontiguousarray(
            enc_r[sl].transpose(0, 2, 1)                     # [8, 1024, 500]
            .reshape(BL, C, 128, R)
            .transpose(0, 2, 1, 3))                          # [8, 128, C, 500]
        hid_c = hidden[sl]
        mask_t = np.ascontiguousarray(
            np.broadcast_to(-0.5 * maskf[sl].T[None, :, :], (128, T, BL)))
        hid_tm = tmajor(hid_c)
        in_maps.append({
            "enc8": _f8(enc_t.reshape(BL, 128, CP, 2, R)),
            "encb": _bf(enc_t),
            "hid8": _f8(hid_tm.reshape(128, CP, 2, BL)),
            "hidT_b": _bf(hid_tm),
            "hidT_f": _f32(hid_tm),
            "xemb8": _f8(tmajor(x_emb[sl]).reshape(128, CP, 2, BL), 8.0),
            "w1w8": w1w, "w2w8": w2w, "vw8": vw,
            "w1u8": w1u, "w2u8": w2u, "vu8": vu,
            "ctxk8": ctxk, "ctxrk8": ctxrk,
            "deckA8": deckA, "deckB8": deckB, "decrk": decrk,
            "qb_w": qb_w, "qb_u64": qb_u64, "cbx": cbx,
            "cb1h_b": cb1h, "dbx": dbx, "db1h_b": db1h,
            "mask_t": _bf(mask_t),
            "ones_b": ones_b, "ident": ident,
        })
    return in_maps


def run(inputs):
    nc = _get_nc()
    in_maps = prepare_in_maps(inputs)
    res = run_bass_kernel_spmd(nc, in_maps, list(range(NCORES)))
    out = np.concatenate([res.results[c]["out"] for c in range(NCORES)], axis=0)
    return np.ascontiguousarray(out.astype(np.float32)), res


def kernel(**inputs):
    out, _ = run(inputs)
    return out, out
